# revision 5
# baseline (speedup 1.0000x reference)
"""Sliding-window attention (window=256) on 8 TRN2 NeuronCores — v2.

Design (vs the fp32r baseline):
- All compute in bf16 (inputs SWDGE-cast on load): every matmul streams at
  1 cycle/row regardless of width, weight loads get FWL.
- Heads processed in PAIRS: head A's transposed Q^T/K^T live on partitions
  0-63, head B's on 64-127 (produced by fused [128,128] PE transposes of
  column-interleaved natural tiles).  The two heads' QK chunk matmuls use
  row groups (0,0)/(64,0) via base-partition-64 APs and run CONCURRENTLY
  on the PE array (contraction is only d=64).
- Transposes are regular matmuls against a bf16 identity (counts as PE-busy
  for the HAM clock gate, unlike transpose-mode), emitted interleaved with
  block compute so the PE never idles into a re-throttle window.
- Scores go to one shared [128,2048] fp32 PSUM tile (4 banks) in 2 waves:
  wave1 = chunks {c0,c2,c1,c4} of both heads, wave2 = {c3,c5} (reuses banks
  after the wave-1 exp drains).  ONE merged exp per wave on ACT
  ([128,2048] then [128,1024]) writing bf16 P^T.
- Band masking = 2 wide bf16 tensor_tensor multiplies against a precomputed
  {1,0} mega-mask (DVE 4x mode: all-SBUF, all-2-byte).
- PV is per-chunk into a [65,512] fp32 PSUM bank per head using has_written
  semantics: the c2 matmul (cols 0:384) is the only start=True (clears the
  bank's has_written bits); every other chunk accumulates where bits are set
  and overwrites where they aren't.  V carries a ones column so row 64
  accumulates the softmax denominator.
- Epilogue: O^T [65,512] is cast-copied to SBUF bf16 [80,512], transposed by
  the DMA XBAR (nc.sync dma transpose) to [128,4,80] (query q = 4p+j), then
  reciprocal + one broadcast multiply (all SBUF) and a contiguous store.
"""

import numpy as np

import concourse.bass as bass
import concourse.mybir as mybir
from concourse import bacc
from concourse.tile import TileContext
from concourse import bass_utils
from concourse.masks import make_identity

dt = mybir.dt

B, H, S, D = 4, 16, 4096, 64
W = 256
N_CORES = 8
BH = (B * H) // N_CORES      # heads per core = 8
NPAIR = BH // 2              # head pairs per core = 4
QT = 512                     # queries per block
NB = S // QT                 # blocks per head = 8
NT = S // 128                # 128-tiles per head = 32
SCALE = float(D) ** -0.5

# chunk c covers key chunk g = 4t-2+c; query window within the block:
QW = {0: (0, 128), 1: (0, 256), 2: (0, 384),
      3: (128, 512), 4: (256, 512), 5: (384, 512)}
W1 = [0, 2, 1, 4]            # wave-1 chunks (banks 0-1 per head)
W2 = [3, 5]                  # wave-2 chunks
ST1_OFF = {0: 0, 2: 128, 1: 512, 4: 768}
ST2_OFF = {3: 0, 5: 384}


def st_off(h, c):
    """Column offset of (head h, chunk c) scores within its wave's
    [128,1024] ST region (wave1 regions are per-head, wave2 is shared)."""
    if c in ST1_OFF:
        return ST1_OFF[c]
    return 512 * h + ST2_OFF[c]


def pt_off(h, c):
    """Column offset of (head h, chunk c) probabilities in the [128,3072] P^T."""
    if c in ST1_OFF:
        return 1024 * h + ST1_OFF[c]
    return 2048 + 512 * h + ST2_OFF[c]


def build_mega_mask():
    """{1,0} float32 [128, 3072] band mask laid out to match pt_off."""
    m = np.zeros((128, 3072), dtype=np.float32)
    kl = np.arange(128)[:, None]
    for h in (0, 1):
        for c in range(6):
            q0, q1 = QW[c]
            qi = np.arange(q0, q1)[None, :]
            key = 128 * (c - 2) + kl
            valid = (key >= qi - 256) & (key <= qi)
            off = pt_off(h, c)
            m[:, off:off + (q1 - q0)] = valid.astype(np.float32)
    return m


def build_core_kernel(n_bh=BH):
    nc = bacc.Bacc("TRN2", target_bir_lowering=False)
    qd = nc.dram_tensor("q", [n_bh * S, D], dt.float32, kind="ExternalInput")
    kd = nc.dram_tensor("k", [n_bh * S, D], dt.float32, kind="ExternalInput")
    vd = nc.dram_tensor("v", [n_bh * S, D], dt.float32, kind="ExternalInput")
    md = nc.dram_tensor("band_mask", [128, 3072], dt.float32,
                        kind="ExternalInput")
    od = nc.dram_tensor("o", [n_bh * S, D], dt.float32, kind="ExternalOutput")

    with TileContext(nc) as tc:
        with (
            tc.tile_pool(name="const", bufs=1) as constp,
            tc.tile_pool(name="bigio", bufs=2) as bigio,
            tc.tile_pool(name="qkt", bufs=2) as qktp,
            tc.tile_pool(name="ptp", bufs=4) as ptp,
            tc.tile_pool(name="work", bufs=3) as work,
            tc.tile_pool(name="psst", bufs=3, space="PSUM") as psst,
            tc.tile_pool(name="psot", bufs=1, space="PSUM") as psot,
        ):
            ident = constp.tile([128, 128], dt.float32)
            make_identity(nc, ident)
            identb = constp.tile([128, 128], dt.bfloat16)
            nc.vector.tensor_copy(identb[:], ident[:])
            mega = constp.tile([128, 3072], dt.bfloat16)
            nc.gpsimd.dma_start(mega[:], md[:])  # cast fp32 -> bf16

            def emit_load_nat(p, dn, tag, t0=0, t1=NT, nat=None):
                """SWDGE cast-load of q or k for pair p (tile range
                [t0, t1)), head-interleaved within each 128-tile's columns."""
                base = 2 * p * S
                if nat is None:
                    nat = bigio.tile([128, NT * 128], dt.bfloat16, tag=tag,
                                     name=f"{tag}{p}")
                dst4 = nat[:].rearrange("p (t h d) -> p t h d", h=2, d=64)
                for h in (0, 1):
                    srcap = dn[base + h * S + 128 * t0:
                               base + h * S + 128 * t1, :].rearrange(
                        "(t p) d -> p t d", p=128)
                    nc.gpsimd.dma_start(dst4[:, t0:t1, h, :], srcap)
                return nat

            def emit_load_v(p):
                base = 2 * p * S
                vtp = bigio.tile([128, NT * 2 * 65], dt.bfloat16, tag="vtp",
                                 name=f"vtp{p}")
                vt4 = vtp[:].rearrange("p (g h e) -> p g h e", h=2, e=65)
                for h in (0, 1):
                    vsrc = vd[base + h * S:base + (h + 1) * S, :].rearrange(
                        "(g p) d -> p g d", p=128)
                    nc.gpsimd.dma_start(vt4[:, :, h, 0:D], vsrc)
                nc.vector.memset(vt4[:, :, :, D], 1.0)
                return vtp

            def alloc_qkt(p):
                qt2 = qktp.tile([128, S], dt.bfloat16, tag="qt2",
                                name=f"qt2_{p}")
                kt2 = qktp.tile([128, S], dt.bfloat16, tag="kt2",
                                name=f"kt2_{p}")
                return qt2, kt2

            def emit_prep(nats, qk2, t):
                """Transpose tiles 4t..4t+3 of q and k for the NEXT pair via
                regular matmuls against the identity: unlike transpose-mode
                these count as PE-busy for the HAM clock gate, keeping the
                array at 2.4GHz.  fp32 PSUM (1 bank) from the st pool
                rotation; copy-out split DVE (q) / ACT (k)."""
                qnat, knat, _ = nats
                qt2, kt2 = qk2
                for nat, dst in ((qnat, qt2), (knat, kt2)):
                    tr = psst.tile([128, 2048], dt.bfloat16, tag="st",
                                   name="tr")
                    for u in range(4):
                        i = 4 * t + u
                        nc.tensor.transpose(
                            tr[:, 128 * u:128 * (u + 1)],
                            nat[:, 128 * i:128 * (i + 1)], identb[:])
                    nc.vector.tensor_copy(dst[:, 512 * t:512 * (t + 1)],
                                          tr[:, 0:512])

            def emit_qk(qk2, st, t, chunks, heads):
                qt2, kt2 = qk2
                for c in chunks:
                    g = 4 * t - 2 + c
                    if g < 0:
                        continue
                    q0, q1 = QW[c]
                    for h in heads:
                        b = 64 * h
                        nc.tensor.matmul(
                            st[:, st_off(h, c):st_off(h, c) + (q1 - q0)],
                            kt2[b:b + 64, 128 * g:128 * (g + 1)],
                            qt2[b:b + 64, QT * t + q0:QT * t + q1],
                            start=True, stop=True)

            def emit_pv(vtp, pt, t):
                vt4 = vtp[:].rearrange("p (g h e) -> p g h e", h=2, e=65)
                chunks = [c for c in range(6) if 4 * t - 2 + c >= 0]
                order = [2] + [c for c in chunks if c != 2]
                ot2 = psot.tile([65, 2 * QT], dt.float32, tag="ot2",
                                name="ot2")
                for idx, c in enumerate(order):
                    g = 4 * t - 2 + c
                    q0, q1 = QW[c]
                    for h in (0, 1):
                        nc.tensor.matmul(
                            ot2[:, QT * h + q0:QT * h + q1],
                            vt4[:, g, h, :],
                            pt[:, pt_off(h, c):pt_off(h, c) + (q1 - q0)],
                            start=(idx == 0), stop=(idx == len(order) - 1),
                            skip_group_check=True)
                return ot2

            def emit_epi_head(ot2):
                """osb cast + dma transpose for both heads (merged)."""
                osb = work.tile([80, 2 * QT], dt.bfloat16, tag="osb",
                                name="osb")
                nc.vector.tensor_copy(osb[0:65, 0:QT], ot2[:, 0:QT])
                nc.scalar.copy(osb[0:65, QT:2 * QT], ot2[:, QT:2 * QT])
                osbT = work.tile([128, 8 * 80], dt.bfloat16,
                                 tag="osbT", name="osbT")
                o3 = osbT[:].rearrange("p (j e) -> p j e", e=80)
                nc.sync.dma_start(o3, osb[:], transpose=True)
                return osbT

            def emit_epi_tail(osbT, p, t):
                o3 = osbT[:].rearrange("p (j e) -> p j e", e=80)
                rc = work.tile([128, 8], dt.float32, tag="rc", name="rc")
                nc.vector.reciprocal(rc[:], o3[:, :, D])
                outsb = work.tile([128, 512], dt.float32, tag="outsb",
                                  name="outsb")
                u3 = outsb[:].rearrange("p (j e) -> p j e", e=64)
                rcb = rc[:].rearrange("p (j o) -> p j o", o=1)
                nc.vector.tensor_tensor(
                    u3, o3[:, :, 0:D], rcb.broadcast_to([128, 8, 64]),
                    op=mybir.AluOpType.mult)
                for h in (0, 1):
                    base = (2 * p + h) * S
                    # dma transpose is j-major: query q = 128j + p
                    dst = od[base + QT * t:base + QT * (t + 1), :].rearrange(
                        "(j p) d -> p j d", p=128)
                    nc.sync.dma_start(dst, u3[:, 4 * h:4 * h + 4, :])

            # ---- prologue: q/k for pairs 0/1 and v for pair 0 up front;
            # later pairs' loads are spread across earlier pairs' blocks to
            # keep the SDMA queue shallow for epilogue DMAs
            q0 = emit_load_nat(0, qd, "qnat", 0, NT // 2)
            k0 = emit_load_nat(0, kd, "knat", 0, NT // 2)
            nats = [[q0, k0, None], [None, None, None]]
            qk2s = [alloc_qkt(0)]
            for t in range(NB // 2):
                emit_prep(nats[0], qk2s[0], t)
            emit_load_nat(0, qd, "qnat", NT // 2, NT, nat=q0)
            emit_load_nat(0, kd, "knat", NT // 2, NT, nat=k0)
            nats[0][2] = emit_load_v(0)
            nats[1][0] = emit_load_nat(1, qd, "qnat")
            nats[1][1] = emit_load_nat(1, kd, "knat")
            for t in range(NB // 2, NB):
                emit_prep(nats[0], qk2s[0], t)

            for p in range(NPAIR):
                if p + 2 < NPAIR:
                    nats.append([None, None, None])
                if p + 1 < NPAIR:
                    qk2s.append(alloc_qkt(p + 1))
                vtp_p = nats[p][2]
                for t in range(NB):
                    if t == 0 and p + 1 < NPAIR and nats[p + 1][2] is None:
                        nats[p + 1][2] = emit_load_v(p + 1)
                    if p + 2 < NPAIR:
                        if t == 2:
                            nats[p + 2][0] = emit_load_nat(p + 2, qd, "qnat")
                        elif t == 4:
                            nats[p + 2][1] = emit_load_nat(p + 2, kd, "knat")
                    pt = ptp.tile([128, 3072], dt.bfloat16, tag="pt",
                                  name="pt")
                    # wave A1: head 0, wave-1 chunks
                    stA = psst.tile([128, 1024], dt.float32, tag="st",
                                    name="st")
                    emit_qk(qk2s[p], stA, t, W1, (0,))
                    nc.scalar.activation(
                        pt[:, 0:1024], stA[:],
                        mybir.ActivationFunctionType.Exp, scale=SCALE)
                    # wave B1: head 1, wave-1 chunks (other region)
                    stB = psst.tile([128, 1024], dt.float32, tag="st",
                                    name="st")
                    emit_qk(qk2s[p], stB, t, W1, (1,))
                    nc.scalar.activation(
                        pt[:, 1024:2048], stB[:],
                        mybir.ActivationFunctionType.Exp, scale=SCALE)
                    if t > 0:
                        ot2 = emit_pv(vtp_p, pt_prev, t - 1)
                        osbT_prev = emit_epi_head(ot2)
                    # wave 2: both heads' {c3, c5} (region = A1's, now free)
                    st2 = psst.tile([128, 1024], dt.float32, tag="st",
                                    name="st")
                    emit_qk(qk2s[p], st2, t, W2, (0, 1))
                    nc.scalar.activation(
                        pt[:, 2048:3072], st2[:],
                        mybir.ActivationFunctionType.Exp, scale=SCALE)
                    if p + 1 < NPAIR:
                        emit_prep(nats[p + 1], qk2s[p + 1], t)
                    nc.vector.tensor_tensor(
                        pt[:, 0:2048], pt[:, 0:2048], mega[:, 0:2048],
                        op=mybir.AluOpType.mult)
                    nc.vector.tensor_tensor(
                        pt[:, 2048:2816], pt[:, 2048:2816],
                        mega[:, 2048:2816], op=mybir.AluOpType.mult)
                    nc.gpsimd.tensor_tensor(
                        pt[:, 2816:3072], pt[:, 2816:3072],
                        mega[:, 2816:3072], op=mybir.AluOpType.mult)
                    if t > 0:
                        emit_epi_tail(osbT_prev, p, t - 1)
                    pt_prev = pt
                ot2 = emit_pv(vtp_p, pt_prev, NB - 1)
                osbT_last = emit_epi_head(ot2)
                emit_epi_tail(osbT_last, p, NB - 1)

    nc.finalize()
    return nc


_NC_CACHE = []


def _get_nc():
    if not _NC_CACHE:
        _NC_CACHE.append(build_core_kernel())
    return _NC_CACHE[0]


def make_in_maps(q, k, v):
    qr = np.ascontiguousarray(np.asarray(q, dtype=np.float32).reshape(B * H, S, D))
    kr = np.ascontiguousarray(np.asarray(k, dtype=np.float32).reshape(B * H, S, D))
    vr = np.ascontiguousarray(np.asarray(v, dtype=np.float32).reshape(B * H, S, D))
    band = np.ascontiguousarray(build_mega_mask())

    in_maps = []
    for i in range(N_CORES):
        in_maps.append({
            "q": np.ascontiguousarray(qr[BH * i:BH * (i + 1)].reshape(BH * S, D)),
            "k": np.ascontiguousarray(kr[BH * i:BH * (i + 1)].reshape(BH * S, D)),
            "v": np.ascontiguousarray(vr[BH * i:BH * (i + 1)].reshape(BH * S, D)),
            "band_mask": band,
        })
    return in_maps


def gather_out(res):
    out = np.empty((B * H, S, D), dtype=np.float32)
    for i in range(N_CORES):
        out[BH * i:BH * (i + 1)] = res.results[i]["o"].reshape(BH, S, D)
    return out.reshape(B, H, S, D)


def kernel(q, k, v):
    nc = _get_nc()
    in_maps = make_in_maps(q, k, v)
    res = bass_utils.run_bass_kernel_spmd(nc, in_maps, core_ids=list(range(N_CORES)))
    return gather_out(res)
